# revision 10
# baseline (speedup 1.0000x reference)
"""Trainium2 Bass kernel for nn_MergeHeads (moe_routing).

Computes out[t] = sum_a p[t,a] * (x[t,a] @ W[idx[t,a]] + b[idx[t,a]])
for B*S = 16384 tokens, A=2 slots, H=8 heads, DH=128, DM=2048.

Strategy: data-parallel over tokens across 8 NeuronCores (2048 tokens
each); W/b replicated.  Per core, dense routed formulation computed
almost entirely on the TensorEngine in float32r (full PE rate at
N>=256, ~1.5e-4 max rel err):

  routing (per 128-token tile, as matmuls):
    G_a[t', (h,t)] = eye[t',t] * wgt[t',h,a]   (one DVE op per slot;
                                                wgt = (idx==h)*p)
    routedT[d, (h,t)] = sum_a x_a^T @ G_a      (4 accumulated matmuls)
  main (per tile):
    y[t, mc] = sum_h routedT[:,h,:].T @ W[h][:,mc] + wsT.T @ b[:,mc]
    (h-outer / mc-inner over 4 PSUM banks so each stationary loads once)

DMA layout: W as 8 contiguous 1MB loads split across both HWDGE rings;
x prefetched 3 tiles ahead on the sync ring; y stores issued from the
scalar ring (1MB per tile) so loads never queue behind stores.
"""

import os
import numpy as np

B, S, A, H, DH, DM = 4, 4096, 2, 8, 128, 2048
NCORES = 8
T = B * S
TLOC = T // NCORES        # 2048 tokens per core
P = 128                   # partitions / token tile
NT = TLOC // P            # 16 token tiles per core
NFREE = 512               # matmul moving free dim (one PSUM bank of fp32)
MC = DM // NFREE          # 4 output chunks per token tile
PREF = 3                  # x-tile prefetch distance

# compute dtype: "f32r" (default), "bf16", or "f32"
CDT_MODE = os.environ.get("TRNK_DTYPE", "f32r")

_CACHE = {}


def _build_nc():
    import concourse.mybir as mybir
    from concourse import bacc
    from concourse.tile import TileContext
    from concourse.masks import make_identity

    f32 = mybir.dt.float32
    cdt = {
        "f32r": mybir.dt.float32r,
        "bf16": mybir.dt.bfloat16,
        "f32": mybir.dt.float32,
    }[CDT_MODE]

    nc = bacc.Bacc("TRN2", target_bir_lowering=False, debug=False)

    x_d = nc.dram_tensor("x", [TLOC, A, DH], f32, kind="ExternalInput")
    idx_d = nc.dram_tensor("idxf", [TLOC, A], f32, kind="ExternalInput")
    p_d = nc.dram_tensor("p", [TLOC, A], f32, kind="ExternalInput")
    w_d = nc.dram_tensor("W", [H, DH, DM], f32, kind="ExternalInput")
    b_d = nc.dram_tensor("b", [H, DM], f32, kind="ExternalInput")
    hg_d = nc.dram_tensor("hgrid", [P, H, A], f32, kind="ExternalInput")
    y_d = nc.dram_tensor("out", [TLOC, DM], f32, kind="ExternalOutput")

    with TileContext(nc) as tc:
        with tc.tile_pool(name="const", bufs=1) as const, \
             tc.tile_pool(name="wstage", bufs=2) as wstage, \
             tc.tile_pool(name="xpool", bufs=PREF + 2) as xpool, \
             tc.tile_pool(name="xrpool", bufs=3) as xrpool, \
             tc.tile_pool(name="gpool", bufs=4) as gpool, \
             tc.tile_pool(name="rpool", bufs=2) as rpool, \
             tc.tile_pool(name="wst", bufs=2) as wstpool, \
             tc.tile_pool(name="ypool", bufs=3) as ypool, \
             tc.tile_pool(name="pr", bufs=3, space="PSUM") as prpool, \
             tc.tile_pool(name="py", bufs=4, space="PSUM") as pypool, \
             tc.tile_pool(name="pw", bufs=1, space="PSUM") as pwpool:

            # ---- constants / setup ----
            eye = const.tile([P, P], f32, tag="eye")
            make_identity(nc, eye[:])

            hg = const.tile([P, H, A], f32, tag="hg")
            nc.sync.dma_start(hg[:], hg_d[:])

            idx_sb = const.tile([P, NT, A], f32, tag="idx")
            p_sb = const.tile([P, NT, A], f32, tag="p")
            # dst[tp, i, a] = src[i*P + tp, a]
            nc.sync.dma_start(idx_sb[:], idx_d[:].rearrange("(i tp) a -> tp i a", tp=P))
            nc.sync.dma_start(p_sb[:], p_d[:].rearrange("(i tp) a -> tp i a", tp=P))

            # W -> SBUF, rounded: W_r[d, h, m]; 1MB contiguous loads
            # split across both HWDGE rings so they drain in parallel.
            w_r = const.tile([P, H, DM], cdt, tag="w_r")
            for h in range(H):
                st = wstage.tile([P, DM], f32, tag="wst")
                (nc.sync if h % 2 == 0 else nc.scalar).dma_start(st[:], w_d[h, :, :])
                if h % 2 == 0:
                    nc.vector.tensor_copy(w_r[:, h, :], st[:])
                else:
                    nc.scalar.copy(w_r[:, h, :], st[:])

            # b -> SBUF rounded: b_r[h, m] (partitions 0..7)
            bstage = const.tile([H, DM], f32, tag="bstage")
            nc.sync.dma_start(bstage[:], b_d[:])
            b_r = const.tile([H, DM], cdt, tag="b_r")
            nc.vector.tensor_copy(b_r[:], bstage[:])

            # routing weights wgt[tp, i, h, a] = (idx==h) * p
            wgt = const.tile([P, NT, H, A], f32, tag="wgt")
            idx_b = idx_sb[:].unsqueeze(2).broadcast_to([P, NT, H, A])
            p_b = p_sb[:].unsqueeze(2).broadcast_to([P, NT, H, A])
            hg_b = hg[:].unsqueeze(1).broadcast_to([P, NT, H, A])
            nc.vector.tensor_tensor(wgt[:], idx_b, hg_b, mybir.AluOpType.is_equal)
            nc.vector.tensor_tensor(wgt[:], wgt[:], p_b, mybir.AluOpType.mult)
            # per-head prob sums wsum[tp, i, h] = wgt[...,0] + wgt[...,1]
            wsum = const.tile([P, NT, H], f32, tag="wsum")
            nc.vector.tensor_tensor(
                wsum[:], wgt[:, :, :, 0], wgt[:, :, :, 1], mybir.AluOpType.add
            )

            eye_b = eye[:].unsqueeze(1).broadcast_to([P, H, P])

            # x tile prefetch
            x_tiles = {}

            def issue_x(i):
                if i < NT and i not in x_tiles:
                    x_t = xpool.tile([P, A, DH], f32, tag="x")
                    nc.sync.dma_start(x_t[:], x_d[i * P:(i + 1) * P, :, :])
                    x_tiles[i] = x_t

            for i in range(PREF):
                issue_x(i)

            # ---- main pipeline, software-pipelined by one tile ----
            prev = None  # (routedT tile, wsT tile) of tile i-1
            for i in range(NT + 1):
                cur = None
                issue_x(i + PREF)
                if i < NT:
                    x_t = x_tiles.pop(i)
                    # round x to compute dtype for the routing matmuls
                    x_r = xrpool.tile([P, A, DH], cdt, tag="xr")
                    nc.vector.tensor_copy(x_r[:], x_t[:])

                    # G_a[t', (h,t)] = eye * wgt (rounded on write)
                    g0 = gpool.tile([P, H, P], cdt, tag="g")
                    g1 = gpool.tile([P, H, P], cdt, tag="g")
                    w0_b = wgt[:, i, :, 0].unsqueeze(2).broadcast_to([P, H, P])
                    w1_b = wgt[:, i, :, 1].unsqueeze(2).broadcast_to([P, H, P])
                    nc.vector.tensor_tensor(g0[:], eye_b, w0_b, mybir.AluOpType.mult)
                    nc.vector.tensor_tensor(g1[:], eye_b, w1_b, mybir.AluOpType.mult)

                    # routedT[d, (h,t)] = sum_a x_a^T @ G_a  (2 PSUM banks)
                    r_t = rpool.tile([P, H, DH], cdt, tag="r")
                    pr0 = prpool.tile([P, NFREE], f32, tag="pr")
                    pr1 = prpool.tile([P, NFREE], f32, tag="pr")
                    g0f = g0[:].rearrange("p h t -> p (h t)")
                    g1f = g1[:].rearrange("p h t -> p (h t)")
                    nc.tensor.matmul(pr0[:], x_r[:, 0, :], g0f[:, 0:NFREE],
                                     start=True, stop=False)
                    nc.tensor.matmul(pr1[:], x_r[:, 0, :], g0f[:, NFREE:2 * NFREE],
                                     start=True, stop=False)
                    nc.tensor.matmul(pr0[:], x_r[:, 1, :], g1f[:, 0:NFREE],
                                     start=False, stop=True)
                    nc.tensor.matmul(pr1[:], x_r[:, 1, :], g1f[:, NFREE:2 * NFREE],
                                     start=False, stop=True)
                    dst0 = r_t[:, 0:4, :].rearrange("p a b -> p (a b)")
                    dst1 = r_t[:, 4:8, :].rearrange("p a b -> p (a b)")
                    nc.vector.tensor_copy(dst0, pr0[:])
                    nc.scalar.copy(dst1, pr1[:])

                    # transposed per-head prob sums for the bias matmul
                    pw_t = pwpool.tile([H, P], f32, tag="pw")
                    nc.tensor.transpose(pw_t[:], wsum[:, i, :], eye[:])
                    ws_t = wstpool.tile([H, P], cdt, tag="ws")
                    nc.scalar.copy(ws_t[:], pw_t[:])
                    cur = (r_t, ws_t)

                if i >= 1:
                    r_p, ws_p = prev
                    j = i - 1
                    # h-outer / mc-inner: each stationary (routedT head, or
                    # the bias wsT) loads once and streams all 4 output
                    # chunks; 4 PSUM banks accumulate concurrently.
                    py_ts = []
                    for _mc in range(MC):
                        py_t = pypool.tile([P, NFREE], f32, tag="py")
                        py_ts.append(py_t)
                    for h in range(H):
                        for mc in range(MC):
                            nc.tensor.matmul(
                                py_ts[mc][:],
                                r_p[:, h, :],
                                w_r[:, h, mc * NFREE:(mc + 1) * NFREE],
                                start=(h == 0), stop=False,
                            )
                    for mc in range(MC):
                        nc.tensor.matmul(
                            py_ts[mc][:], ws_p[:],
                            b_r[:, mc * NFREE:(mc + 1) * NFREE],
                            start=False, stop=True,
                        )
                    y_t = ypool.tile([P, DM], f32, tag="y")
                    for mc in range(MC):
                        dst = y_t[:, mc * NFREE:(mc + 1) * NFREE]
                        if mc % 2 == 0:
                            nc.vector.tensor_copy(dst, py_ts[mc][:])
                        else:
                            nc.scalar.copy(dst, py_ts[mc][:])
                    # store from the scalar HWDGE ring: loads (sync ring)
                    # never queue behind stores.
                    nc.scalar.dma_start(y_d[j * P:(j + 1) * P, :], y_t[:])
                prev = cur

    nc.compile()
    return nc


def _get_nc():
    if "nc" not in _CACHE:
        _CACHE["nc"] = _build_nc()
    return _CACHE["nc"]


def kernel(embedding, sel_idx, sel_probs, W, b):
    from concourse.bass_utils import run_bass_kernel_spmd

    emb = np.ascontiguousarray(embedding, dtype=np.float32).reshape(T, A, DH)
    idxf = np.ascontiguousarray(sel_idx).reshape(T, A).astype(np.float32)
    pf = np.ascontiguousarray(sel_probs, dtype=np.float32).reshape(T, A)
    Wf = np.ascontiguousarray(W, dtype=np.float32)
    bf = np.ascontiguousarray(b, dtype=np.float32)
    hgrid = np.ascontiguousarray(
        np.broadcast_to(
            np.arange(H, dtype=np.float32)[None, :, None], (P, H, A)
        )
    )

    nc = _get_nc()
    in_maps = []
    for c in range(NCORES):
        sl = slice(c * TLOC, (c + 1) * TLOC)
        in_maps.append({
            "x": emb[sl],
            "idxf": idxf[sl],
            "p": pf[sl],
            "W": Wf,
            "b": bf,
            "hgrid": hgrid,
        })

    trace = os.environ.get("TRNK_TRACE") == "1"
    if trace:
        _register_ntff_stub()
    res = run_bass_kernel_spmd(
        nc, in_maps, core_ids=list(range(NCORES)), trace=trace
    )
    if trace:
        _CACHE["exec_time_ns"] = res.exec_time_ns
        _CACHE["results_obj"] = res

    out = np.concatenate(
        [res.results[c]["out"] for c in range(NCORES)], axis=0
    )
    return out.reshape(B, S, DM)


def _register_ntff_stub():
    """antenv.axon_hooks is absent in this image; back it with the boot
    ctypes NTFF hook so trace=True works under axon."""
    import sys, types
    try:
        import antenv.axon_hooks  # noqa: F401
        return
    except ImportError:
        pass
    try:
        import antenv
        from trn_agent_boot.trn_boot import _ntff_profile_via_ctypes
    except ImportError:
        return
    mod = types.ModuleType("antenv.axon_hooks")
    hook = [None]

    def set_axon_ntff_profile_hook(h):
        hook[0] = h

    def get_axon_ntff_profile_hook():
        if hook[0] is None:
            hook[0] = _ntff_profile_via_ctypes("/opt/axon/libaxon_pjrt.so")
        return hook[0]

    mod.set_axon_ntff_profile_hook = set_axon_ntff_profile_hook
    mod.get_axon_ntff_profile_hook = get_axon_ntff_profile_hook
    sys.modules["antenv.axon_hooks"] = mod
    antenv.axon_hooks = mod


# revision 11
# speedup vs baseline: 1.0072x; 1.0072x over previous
"""Trainium2 Bass kernel for nn_MergeHeads (moe_routing).

Computes out[t] = sum_a p[t,a] * (x[t,a] @ W[idx[t,a]] + b[idx[t,a]])
for B*S = 16384 tokens, A=2 slots, H=8 heads, DH=128, DM=2048.

Strategy: data-parallel over tokens across 8 NeuronCores (2048 tokens
each); W/b replicated.  Per core, dense routed formulation computed
almost entirely on the TensorEngine in float32r (full PE rate at
N>=256, ~1.5e-4 max rel err):

  routing (per 128-token tile, as matmuls):
    G_a[t', (h,t)] = eye[t',t] * wgt[t',h,a]   (one DVE op per slot;
                                                wgt = (idx==h)*p)
    routedT[d, (h,t)] = sum_a x_a^T @ G_a      (4 accumulated matmuls)
  main (per tile):
    y[t, mc] = sum_h routedT[:,h,:].T @ W[h][:,mc] + wsT.T @ b[:,mc]
    (h-outer / mc-inner over 4 PSUM banks so each stationary loads once)

DMA layout: W as 8 contiguous 1MB loads split across both HWDGE rings;
x prefetched 3 tiles ahead on the sync ring; y stores issued from the
scalar ring (1MB per tile) so loads never queue behind stores.
"""

import os
import numpy as np

B, S, A, H, DH, DM = 4, 4096, 2, 8, 128, 2048
NCORES = 8
T = B * S
TLOC = T // NCORES        # 2048 tokens per core
P = 128                   # partitions / token tile
NT = TLOC // P            # 16 token tiles per core
NFREE = 512               # matmul moving free dim (one PSUM bank of fp32)
MC = DM // NFREE          # 4 output chunks per token tile
PREF = 3                  # x-tile prefetch distance (ahead of routing)
DEPTH = 8                 # routing runs this many tiles ahead of mains

# compute dtype: "f32r" (default), "bf16", or "f32"
CDT_MODE = os.environ.get("TRNK_DTYPE", "f32r")

_CACHE = {}


def _build_nc():
    import concourse.mybir as mybir
    from concourse import bacc
    from concourse.tile import TileContext
    from concourse.masks import make_identity

    f32 = mybir.dt.float32
    cdt = {
        "f32r": mybir.dt.float32r,
        "bf16": mybir.dt.bfloat16,
        "f32": mybir.dt.float32,
    }[CDT_MODE]

    nc = bacc.Bacc("TRN2", target_bir_lowering=False, debug=False)

    x_d = nc.dram_tensor("x", [TLOC, A, DH], f32, kind="ExternalInput")
    idx_d = nc.dram_tensor("idxf", [TLOC, A], f32, kind="ExternalInput")
    p_d = nc.dram_tensor("p", [TLOC, A], f32, kind="ExternalInput")
    w_d = nc.dram_tensor("W", [H, DH, DM], f32, kind="ExternalInput")
    b_d = nc.dram_tensor("b", [H, DM], f32, kind="ExternalInput")
    hg_d = nc.dram_tensor("hgrid", [P, H, A], f32, kind="ExternalInput")
    y_d = nc.dram_tensor("out", [TLOC, DM], f32, kind="ExternalOutput")

    with TileContext(nc) as tc:
        with tc.tile_pool(name="const", bufs=1) as const, \
             tc.tile_pool(name="wstage", bufs=2) as wstage, \
             tc.tile_pool(name="xpool", bufs=PREF + 2) as xpool, \
             tc.tile_pool(name="xrpool", bufs=3) as xrpool, \
             tc.tile_pool(name="gpool", bufs=4) as gpool, \
             tc.tile_pool(name="rpool", bufs=DEPTH + 1) as rpool, \
             tc.tile_pool(name="wst", bufs=DEPTH + 1) as wstpool, \
             tc.tile_pool(name="ypool", bufs=3) as ypool, \
             tc.tile_pool(name="pr", bufs=3, space="PSUM") as prpool, \
             tc.tile_pool(name="py", bufs=4, space="PSUM") as pypool, \
             tc.tile_pool(name="pw", bufs=1, space="PSUM") as pwpool:

            # ---- constants / setup ----
            eye = const.tile([P, P], f32, tag="eye")
            make_identity(nc, eye[:])

            hg = const.tile([P, H, A], f32, tag="hg")
            nc.sync.dma_start(hg[:], hg_d[:])

            idx_sb = const.tile([P, NT, A], f32, tag="idx")
            p_sb = const.tile([P, NT, A], f32, tag="p")
            # dst[tp, i, a] = src[i*P + tp, a]
            nc.sync.dma_start(idx_sb[:], idx_d[:].rearrange("(i tp) a -> tp i a", tp=P))
            nc.sync.dma_start(p_sb[:], p_d[:].rearrange("(i tp) a -> tp i a", tp=P))

            # W -> SBUF, rounded: W_r[d, h, m]; 1MB contiguous loads
            # split across both HWDGE rings so they drain in parallel.
            w_r = const.tile([P, H, DM], cdt, tag="w_r")
            for h in range(H):
                st = wstage.tile([P, DM], f32, tag="wst")
                (nc.sync if h % 2 == 0 else nc.scalar).dma_start(st[:], w_d[h, :, :])
                if h % 2 == 0:
                    nc.vector.tensor_copy(w_r[:, h, :], st[:])
                else:
                    nc.scalar.copy(w_r[:, h, :], st[:])

            # b -> SBUF rounded: b_r[h, m] (partitions 0..7)
            bstage = const.tile([H, DM], f32, tag="bstage")
            nc.sync.dma_start(bstage[:], b_d[:])
            b_r = const.tile([H, DM], cdt, tag="b_r")
            nc.vector.tensor_copy(b_r[:], bstage[:])

            # routing weights wgt[tp, i, h, a] = (idx==h) * p
            wgt = const.tile([P, NT, H, A], f32, tag="wgt")
            idx_b = idx_sb[:].unsqueeze(2).broadcast_to([P, NT, H, A])
            p_b = p_sb[:].unsqueeze(2).broadcast_to([P, NT, H, A])
            hg_b = hg[:].unsqueeze(1).broadcast_to([P, NT, H, A])
            nc.vector.tensor_tensor(wgt[:], idx_b, hg_b, mybir.AluOpType.is_equal)
            nc.vector.tensor_tensor(wgt[:], wgt[:], p_b, mybir.AluOpType.mult)
            # per-head prob sums wsum[tp, i, h] = wgt[...,0] + wgt[...,1]
            wsum = const.tile([P, NT, H], f32, tag="wsum")
            nc.vector.tensor_tensor(
                wsum[:], wgt[:, :, :, 0], wgt[:, :, :, 1], mybir.AluOpType.add
            )

            eye_b = eye[:].unsqueeze(1).broadcast_to([P, H, P])

            # x tile prefetch
            x_tiles = {}

            def issue_x(i):
                if i < NT and i not in x_tiles:
                    x_t = xpool.tile([P, A, DH], f32, tag="x")
                    nc.sync.dma_start(x_t[:], x_d[i * P:(i + 1) * P, :, :])
                    x_tiles[i] = x_t

            for i in range(PREF):
                issue_x(i)

            # ---- main pipeline: routing runs DEPTH tiles ahead ----
            pending = {}  # tile idx -> (routedT tile, wsT tile)
            for i in range(NT + DEPTH):
                issue_x(i + PREF)
                if i < NT:
                    x_t = x_tiles.pop(i)
                    # round x to compute dtype for the routing matmuls
                    x_r = xrpool.tile([P, A, DH], cdt, tag="xr")
                    nc.vector.tensor_copy(x_r[:], x_t[:])

                    # G_a[t', (h,t)] = eye * wgt (rounded on write)
                    g0 = gpool.tile([P, H, P], cdt, tag="g")
                    g1 = gpool.tile([P, H, P], cdt, tag="g")
                    w0_b = wgt[:, i, :, 0].unsqueeze(2).broadcast_to([P, H, P])
                    w1_b = wgt[:, i, :, 1].unsqueeze(2).broadcast_to([P, H, P])
                    nc.vector.tensor_tensor(g0[:], eye_b, w0_b, mybir.AluOpType.mult)
                    nc.gpsimd.tensor_tensor(g1[:], eye_b, w1_b, mybir.AluOpType.mult)

                    # routedT[d, (h,t)] = sum_a x_a^T @ G_a  (2 PSUM banks)
                    r_t = rpool.tile([P, H, DH], cdt, tag="r")
                    pr0 = prpool.tile([P, NFREE], f32, tag="pr")
                    pr1 = prpool.tile([P, NFREE], f32, tag="pr")
                    g0f = g0[:].rearrange("p h t -> p (h t)")
                    g1f = g1[:].rearrange("p h t -> p (h t)")
                    nc.tensor.matmul(pr0[:], x_r[:, 0, :], g0f[:, 0:NFREE],
                                     start=True, stop=False)
                    nc.tensor.matmul(pr1[:], x_r[:, 0, :], g0f[:, NFREE:2 * NFREE],
                                     start=True, stop=False)
                    nc.tensor.matmul(pr0[:], x_r[:, 1, :], g1f[:, 0:NFREE],
                                     start=False, stop=True)
                    nc.tensor.matmul(pr1[:], x_r[:, 1, :], g1f[:, NFREE:2 * NFREE],
                                     start=False, stop=True)
                    dst0 = r_t[:, 0:4, :].rearrange("p a b -> p (a b)")
                    dst1 = r_t[:, 4:8, :].rearrange("p a b -> p (a b)")
                    nc.vector.tensor_copy(dst0, pr0[:])
                    nc.scalar.copy(dst1, pr1[:])

                    # transposed per-head prob sums for the bias matmul
                    pw_t = pwpool.tile([H, P], f32, tag="pw")
                    nc.tensor.transpose(pw_t[:], wsum[:, i, :], eye[:])
                    ws_t = wstpool.tile([H, P], cdt, tag="ws")
                    nc.scalar.copy(ws_t[:], pw_t[:])
                    pending[i] = (r_t, ws_t)

                if i >= DEPTH:
                    j = i - DEPTH
                    r_p, ws_p = pending.pop(j)
                    # h-outer / mc-inner: each stationary (routedT head, or
                    # the bias wsT) loads once and streams all 4 output
                    # chunks; 4 PSUM banks accumulate concurrently.
                    py_ts = []
                    for _mc in range(MC):
                        py_t = pypool.tile([P, NFREE], f32, tag="py")
                        py_ts.append(py_t)
                    for h in range(H):
                        for mc in range(MC):
                            nc.tensor.matmul(
                                py_ts[mc][:],
                                r_p[:, h, :],
                                w_r[:, h, mc * NFREE:(mc + 1) * NFREE],
                                start=(h == 0), stop=False,
                            )
                    for mc in range(MC):
                        nc.tensor.matmul(
                            py_ts[mc][:], ws_p[:],
                            b_r[:, mc * NFREE:(mc + 1) * NFREE],
                            start=False, stop=True,
                        )
                    y_t = ypool.tile([P, DM], f32, tag="y")
                    for mc in range(MC):
                        dst = y_t[:, mc * NFREE:(mc + 1) * NFREE]
                        if mc % 2 == 0:
                            nc.vector.tensor_copy(dst, py_ts[mc][:])
                        else:
                            nc.scalar.copy(dst, py_ts[mc][:])
                    # store from the scalar HWDGE ring: loads (sync ring)
                    # never queue behind stores.
                    nc.scalar.dma_start(y_d[j * P:(j + 1) * P, :], y_t[:])

    nc.compile()
    return nc


def _get_nc():
    if "nc" not in _CACHE:
        _CACHE["nc"] = _build_nc()
    return _CACHE["nc"]


def kernel(embedding, sel_idx, sel_probs, W, b):
    from concourse.bass_utils import run_bass_kernel_spmd

    emb = np.ascontiguousarray(embedding, dtype=np.float32).reshape(T, A, DH)
    idxf = np.ascontiguousarray(sel_idx).reshape(T, A).astype(np.float32)
    pf = np.ascontiguousarray(sel_probs, dtype=np.float32).reshape(T, A)
    Wf = np.ascontiguousarray(W, dtype=np.float32)
    bf = np.ascontiguousarray(b, dtype=np.float32)
    hgrid = np.ascontiguousarray(
        np.broadcast_to(
            np.arange(H, dtype=np.float32)[None, :, None], (P, H, A)
        )
    )

    nc = _get_nc()
    in_maps = []
    for c in range(NCORES):
        sl = slice(c * TLOC, (c + 1) * TLOC)
        in_maps.append({
            "x": emb[sl],
            "idxf": idxf[sl],
            "p": pf[sl],
            "W": Wf,
            "b": bf,
            "hgrid": hgrid,
        })

    trace = os.environ.get("TRNK_TRACE") == "1"
    if trace:
        _register_ntff_stub()
    res = run_bass_kernel_spmd(
        nc, in_maps, core_ids=list(range(NCORES)), trace=trace
    )
    if trace:
        _CACHE["exec_time_ns"] = res.exec_time_ns
        _CACHE["results_obj"] = res

    out = np.concatenate(
        [res.results[c]["out"] for c in range(NCORES)], axis=0
    )
    return out.reshape(B, S, DM)


def _register_ntff_stub():
    """antenv.axon_hooks is absent in this image; back it with the boot
    ctypes NTFF hook so trace=True works under axon."""
    import sys, types
    try:
        import antenv.axon_hooks  # noqa: F401
        return
    except ImportError:
        pass
    try:
        import antenv
        from trn_agent_boot.trn_boot import _ntff_profile_via_ctypes
    except ImportError:
        return
    mod = types.ModuleType("antenv.axon_hooks")
    hook = [None]

    def set_axon_ntff_profile_hook(h):
        hook[0] = h

    def get_axon_ntff_profile_hook():
        if hook[0] is None:
            hook[0] = _ntff_profile_via_ctypes("/opt/axon/libaxon_pjrt.so")
        return hook[0]

    mod.set_axon_ntff_profile_hook = set_axon_ntff_profile_hook
    mod.get_axon_ntff_profile_hook = get_axon_ntff_profile_hook
    sys.modules["antenv.axon_hooks"] = mod
    antenv.axon_hooks = mod
